# revision 32
# baseline (speedup 1.0000x reference)
"""Trainium2 Bass kernel: BatchNorm -> 2-layer LSTM (32 steps, constant layer-0
input) -> Linear, data-parallel over batch across 8 NeuronCores.

v2 layout strategy (per core, batch shard = 128 rows):
  - All gate matmuls are out[b, j] = lhsT.T @ rhs with lhsT = h^T chunks
    (stationary) and rhs = W^T chunks (moving), fp32 PSUM accumulation.
  - fp8 DoubleRow (K=256/pass) for W_hh0, W_hh1 and the i,f,o gate columns of
    W_ih1; the g-gate column block of W_ih1 stays bf16 (tanh path feeds c
    linearly and dominates the error budget).
  - Biases enter via DVE adds against DMA-replicated bias rows (no K=1 bias
    matmuls on the PE).
  - The final linear y_t = h1_t @ W_lin^T + b_lin is computed inline each step
    (N=256 chunks) instead of a deferred phase; no h1 HBM roundtrip.
  - Init: BN stats batched across all 8 feature chunks, W_ih0 streamed once,
    weight DMAs ordered by first-use time.
"""

import os
import sys

sys.path.insert(0, "/opt/trn_rl_repo")

import numpy as np
import ml_dtypes

import concourse.bass as bass
import concourse.bacc as bacc
import concourse.tile as tile
import concourse.mybir as mybir
from concourse import bass_utils
from concourse.masks import make_identity

BF16 = mybir.dt.bfloat16
FP8 = mybir.dt.float8e4
F32 = mybir.dt.float32
AF = mybir.ActivationFunctionType
ALU = mybir.AluOpType
PM = mybir.MatmulPerfMode

# fp8 scaling: recurrent weights x64, h state x32 -> gate PSUM lands x2048.
# bf16 weights/biases are pre-scaled x2048 on host so all gate contributions
# agree; activations fold in 1/2048 via their scale argument.
WS = 64.0
HS = 32.0
GSC = WS * HS
INV_GSC = 1.0 / GSC

B = 1024          # batch
D = 1024          # hidden = input size
H4 = 4 * D        # gate width
BAR = int(os.environ.get("KERNEL_NSTEPS", "32"))
NCORES = int(os.environ.get("KERNEL_NCORES", "8"))
BS = B // 8       # batch shard per core
EPS = 1e-5
KC = D // 128     # contraction chunks (8)
NG = 4            # gate column groups per layer
GS = D // NG      # group size in hidden cols (256)


def _np_bf16(a):
    return np.ascontiguousarray(a).astype(ml_dtypes.bfloat16)


def _np_fp8(a):
    return np.ascontiguousarray(a).astype(ml_dtypes.float8_e4m3)


GPERM = [0, 1, 3, 2]  # device gate order i, f, o, g (one wide sigmoid over 0:3)


def _gate_perm(wT):
    # wT is W.T with 4 gate blocks of D columns; reorder blocks to GPERM
    return wT.reshape(D, 4, D)[:, GPERM, :]


def _gate_bias(b):
    # [4H] -> [16, 256] tile layout, group-major, gate order GPERM
    return (np.asarray(b, np.float32).reshape(4, 4, 256)[GPERM]
            .transpose(1, 0, 2).reshape(16, 256))


def build_module(nsteps=BAR):
    nc = bacc.Bacc(
        "TRN2",
        target_bir_lowering=False,
        debug=False,
        enable_asserts=False,
        num_devices=NCORES,
        dynamic_dma_scratch_size=512,
    )

    # ---- DRAM I/O -------------------------------------------------------
    d_zT = nc.dram_tensor("zT", [D, B], BF16, kind="ExternalInput").ap()
    d_zTs = nc.dram_tensor("zTs", [D, BS], F32, kind="ExternalInput").ap()
    d_gT = nc.dram_tensor("gammaT", [128, KC], F32, kind="ExternalInput").ap()
    d_bT = nc.dram_tensor("betaT", [128, KC], F32, kind="ExternalInput").ap()
    d_wih0 = nc.dram_tensor("wt_ih0", [D, H4], BF16, kind="ExternalInput").ap()
    d_whh0 = nc.dram_tensor("wt_hh0", [D, H4], FP8, kind="ExternalInput").ap()
    d_wih1_8 = nc.dram_tensor("wt_ih1_8", [D, 3 * D], FP8,
                              kind="ExternalInput").ap()
    d_wih1_g = nc.dram_tensor("wt_ih1_g", [D, D], BF16,
                              kind="ExternalInput").ap()
    d_whh1 = nc.dram_tensor("wt_hh1", [D, H4], FP8, kind="ExternalInput").ap()
    d_wlin = nc.dram_tensor("wt_lin", [D, D], BF16, kind="ExternalInput").ap()
    d_b0 = nc.dram_tensor("b0r", [H4], BF16, kind="ExternalInput").ap()
    d_b1 = nc.dram_tensor("b1r", [H4], BF16, kind="ExternalInput").ap()
    d_blin = nc.dram_tensor("b_lin", [D], F32, kind="ExternalInput").ap()
    d_out = nc.dram_tensor("out", [BS, nsteps, D], F32, kind="ExternalOutput").ap()
    dbg = {}
    if os.environ.get("KERNEL_DEBUG"):
        dbg["znT"] = nc.dram_tensor("dbg_znT", [128, KC, 128], F32,
                                    kind="ExternalOutput").ap()
        dbg["c0i"] = nc.dram_tensor("dbg_c0i", [128, D], F32,
                                    kind="ExternalOutput").ap()
        dbg["x0"] = nc.dram_tensor("dbg_x0", [4, 128, 4, 256], BF16,
                                   kind="ExternalOutput").ap()
        dbg["h0T1"] = nc.dram_tensor("dbg_h0T1", [128, KC, 128], BF16,
                                     kind="ExternalOutput").ap()
        dbg["c01"] = nc.dram_tensor("dbg_c01", [128, D], F32,
                                    kind="ExternalOutput").ap()
        dbg["h1T1"] = nc.dram_tensor("dbg_h1T1", [128, KC, 128], BF16,
                                     kind="ExternalOutput").ap()
        dbg["c11"] = nc.dram_tensor("dbg_c11", [128, D], F32,
                                    kind="ExternalOutput").ap()

    with tile.TileContext(nc) as tc:
        build_body(nc, tc, nsteps,
                   d_zT, d_zTs, d_gT, d_bT,
                   d_wih0, d_whh0, d_wih1_8, d_wih1_g, d_whh1, d_wlin,
                   d_b0, d_b1, d_blin, d_out, dbg)
    nc.compile()
    return nc


def build_body(nc, tc, nsteps, d_zT, d_zTs, d_gT, d_bT,
               d_wih0, d_whh0, d_wih1_8, d_wih1_g, d_whh1, d_wlin,
               d_b0, d_b1, d_blin, d_out, dbg):
    # ---- whole-life SBUF ------------------------------------------------
    life = tc.alloc_tile_pool(name="life", bufs=1)
    whh0 = life.tile([128, KC, H4], FP8, tag="whh0")
    whh1 = life.tile([128, KC, H4], FP8, tag="whh1")
    wih1_8 = life.tile([128, KC, 3 * D], FP8, tag="wih1_8")
    c0 = life.tile([128, D], F32, tag="c0")
    c1 = life.tile([128, D], F32, tag="c1")
    h0T = life.tile([128, KC, 128], BF16, tag="h0T")
    h0T8 = life.tile([128, KC, 128], FP8, tag="h0T8")
    x0sb = life.tile([128, NG, 4, GS], BF16, tag="x0sb")
    idbf = life.tile([128, 128], BF16, tag="idbf")

    h1Tp = tc.alloc_tile_pool(name="h1Tp", bufs=2)
    h1T8p = tc.alloc_tile_pool(name="h1T8p", bufs=2)
    h1T8_init = h1T8p.tile([128, KC, 128], FP8, tag="h1T8", name="h1T8_init")

    make_identity(nc, idbf)

    # ---- INIT phase: BN stats + zn^T + c0/c1 + x0_proj ------------------
    with tc.tile_pool(name="initp", bufs=8) as initp, \
         tc.tile_pool(name="wkp", bufs=3) as wkp, \
         tc.tile_pool(name="ismall", bufs=1) as ismall, \
         tc.tile_pool(name="ipsum", bufs=4, space="PSUM") as ipsum:

        # -- DMA issue order: stats inputs, then weights by first use.
        # Big weights go on the Sync queue; the wk stream and lower-priority
        # loads go on the GpSimd queue so neither blocks the other. --
        zs_all = ismall.tile([128, KC, BS], F32, tag="zs")
        zs_b = bass.AP(tensor=d_zTs.tensor, offset=d_zTs.offset,
                       ap=[[BS, 128], [128 * BS, KC], [1, BS]])
        nc.sync.dma_start(out=zs_all, in_=zs_b)
        gT = ismall.tile([128, KC], F32, tag="gT")
        nc.sync.dma_start(out=gT, in_=d_gT)
        bT = ismall.tile([128, KC], F32, tag="bT")
        nc.sync.dma_start(out=bT, in_=d_bT)
        zts = []
        for k in range(KC):
            zt = initp.tile([128, B], BF16, tag="zt", name=f"zt{k}")
            nc.sync.dma_start(out=zt, in_=d_zT[k * 128:(k + 1) * 128, :])
            zts.append(zt)
        for k in range(KC):
            nc.sync.dma_start(out=whh0[:, k, :],
                              in_=d_whh0[k * 128:(k + 1) * 128, :])
        b0rep = ismall.tile([128, 16, 256], BF16, tag="b0rep")
        b0_b = bass.AP(tensor=d_b0.tensor, offset=d_b0.offset,
                       ap=[[0, 128], [1, H4]])
        nc.sync.dma_start(out=b0rep, in_=b0_b)

        eps_t = ismall.tile([128, 1], F32, tag="eps")
        nc.vector.memset(eps_t, EPS)
        znf = ismall.tile([128, KC, 128], F32, tag="znf")
        idf32 = ismall.tile([128, 128], F32, tag="idf32")
        make_identity(nc, idf32)

        st = ismall.tile([128, KC, 2, 6], F32, tag="st")
        mv = ismall.tile([128, KC, 2], F32, tag="mv")
        sd8 = ismall.tile([128, KC], F32, tag="sd8")
        rs8 = ismall.tile([128, KC], F32, tag="rs8")
        sc8 = ismall.tile([128, KC], F32, tag="sc8")

        for k in range(KC):
            nc.vector.bn_stats(out=st[:, k, 0, :], in_=zts[k][:, 0:512])
            nc.vector.bn_stats(out=st[:, k, 1, :], in_=zts[k][:, 512:1024])
            nc.vector.bn_aggr(out=mv[:, k, :], in_=st[:, k])
        nc.scalar.activation(out=sd8, in_=mv[:, :, 1:2], func=AF.Sqrt,
                             bias=eps_t)
        nc.vector.reciprocal(out=rs8, in_=sd8)
        nc.vector.tensor_mul(sc8, gT, rs8)

        for k in range(KC):
            # zn^T chunk (fp32): (z - mean) * scale + beta
            nc.vector.tensor_scalar(
                out=znf[:, k, :], in0=zs_all[:, k, :],
                scalar1=mv[:, k, 0:1], scalar2=sc8[:, k:k + 1],
                op0=ALU.subtract, op1=ALU.mult)
            nc.vector.tensor_scalar_add(znf[:, k, :], znf[:, k, :],
                                        bT[:, k:k + 1])
            # bf16 copy for matmul lhsT (h0 initial state) + fp8 x32 copies
            # (fp8 must be produced from fp32 — bf16->fp8 converts are broken)
            nc.scalar.copy(out=h0T[:, k, :], in_=znf[:, k, :])
            nc.vector.tensor_scalar_mul(h0T8[:, k, :], znf[:, k, :], HS)
            nc.vector.tensor_scalar_mul(h1T8_init[:, k, :], znf[:, k, :], HS)

        if dbg:
            nc.sync.dma_start(out=dbg["znT"], in_=znf)

        # x0_proj = zn @ W_ih0^T + (b_ih0 + b_hh0), group-major bf16.
        # W_ih0 streamed once; all 4 group PSUMs live (8 banks).
        psg = [ipsum.tile([128, 4, GS], F32, tag="ips", name=f"ips{g}")
               for g in range(NG)]
        for k in range(KC):
            wk = wkp.tile([128, H4], BF16, tag="wi0")
            nc.sync.dma_start(out=wk, in_=d_wih0[k * 128:(k + 1) * 128, :])
            for g in range(NG):
                for q in range(4):
                    nc.tensor.matmul(
                        psg[g][:, q, :], h0T[:, k, :],
                        wk[:, q * D + g * GS:q * D + (g + 1) * GS],
                        start=(k == 0 and q in (0, 2)),
                        stop=(k == KC - 1),
                        skip_group_check=True)
        for g in range(NG):
            nc.vector.tensor_add(x0sb[:, g], psg[g], b0rep[:, 4 * g:4 * g + 4, :])
            if dbg:
                nc.sync.dma_start(out=dbg["x0"][g], in_=x0sb[:, g])

        # whh1/wih1_8 issued after the wk stream (needed from step 0's L1,
        # ~10us later than the last wk chunk)
        for k in range(KC):
            nc.sync.dma_start(out=whh1[:, k, :],
                              in_=d_whh1[k * 128:(k + 1) * 128, :])
        for k in range(KC):
            nc.sync.dma_start(out=wih1_8[:, k, :],
                              in_=d_wih1_8[k * 128:(k + 1) * 128, :])

        # c0 = c1 = zn in [b, d] layout via PE transpose of fp32 zn^T
        # (after x0proj so the transposes don't stall the PE early)
        for k in range(KC):
            pt = ipsum.tile([128, 4, GS], F32, tag="ips", name=f"tpz{k}")
            nc.tensor.transpose(pt[:, 0, 0:128], znf[:, k, :], idf32)
            nc.scalar.copy(out=c0[:, k * 128:(k + 1) * 128], in_=pt[:, 0, 0:128])
        nc.vector.tensor_copy(out=c1, in_=c0)
        if dbg:
            nc.sync.dma_start(out=dbg["c0i"], in_=c0)

    # ---- post-init loads (reuse init SBUF space) ------------------------
    post = tc.alloc_tile_pool(name="post", bufs=1)
    b1rep = post.tile([128, 16, 256], BF16, tag="b1rep")
    b1_b = bass.AP(tensor=d_b1.tensor, offset=d_b1.offset,
                   ap=[[0, 128], [1, H4]])
    nc.sync.dma_start(out=b1rep, in_=b1_b)
    blin = post.tile([128, D], F32, tag="blin")
    blin_b = bass.AP(tensor=d_blin.tensor, offset=d_blin.offset,
                     ap=[[0, 128], [1, D]])
    nc.sync.dma_start(out=blin, in_=blin_b)
    wih1_g = post.tile([128, KC, D], BF16, tag="wih1_g")
    for k in range(KC):
        nc.sync.dma_start(out=wih1_g[:, k, :],
                          in_=d_wih1_g[k * 128:(k + 1) * 128, :])
    wlin = post.tile([128, KC, D], BF16, tag="wlin")
    for k in range(KC):
        nc.sync.dma_start(out=wlin[:, k, :],
                          in_=d_wlin[k * 128:(k + 1) * 128, :])

    # ---- recurrent loop --------------------------------------------------
    with tc.tile_pool(name="gates", bufs=3, space="PSUM") as gpool, \
         tc.tile_pool(name="trp", bufs=1, space="PSUM") as trpool, \
         tc.tile_pool(name="yps", bufs=1, space="PSUM") as ypool, \
         tc.tile_pool(name="tmp", bufs=3) as tmp, \
         tc.tile_pool(name="hfp", bufs=2) as hfp, \
         tc.tile_pool(name="hst", bufs=4) as hst, \
         tc.tile_pool(name="ysp", bufs=2) as ysp:

        h1T8 = h1T8_init

        def cell_math(g, ps, c, x0_slice, hs):
            # gates: ps[:, 0..3, :] = i, f, o, g (pre-activation x GSC, fp32)
            # psum + sbuf -> sbuf: releases the PSUM tile after one DVE op
            src = tmp.tile([128, 4, GS], F32, tag="sum")
            nc.vector.tensor_add(src, ps, x0_slice)
            sg = tmp.tile([128, 3, GS], BF16, tag="sg")
            nc.scalar.activation(out=sg, in_=src[:, 0:3, :], func=AF.Sigmoid,
                                 scale=INV_GSC)
            tg = tmp.tile([128, GS], BF16, tag="tg")
            nc.scalar.activation(out=tg, in_=src[:, 3, :], func=AF.Tanh,
                                 scale=INV_GSC)
            csl = c[:, g * GS:(g + 1) * GS]
            nc.vector.tensor_mul(csl, csl, sg[:, 1, :])          # c *= sig(f)
            tp = tmp.tile([128, GS], F32, tag="tp")
            nc.vector.tensor_mul(tp, sg[:, 0, :], tg)            # sig(i)*tanh(g)
            nc.vector.tensor_add(csl, csl, tp)
            tc2 = tmp.tile([128, GS], BF16, tag="tc2")
            nc.scalar.activation(out=tc2, in_=csl, func=AF.Tanh)
            nc.vector.tensor_mul(hs, sg[:, 2, :], tc2)           # h = sig(o)*tanh(c)

        def hh_matmuls(ps, hT8, w, g):
            for kp in range(KC // 2):
                for q in range(4):
                    nc.tensor.matmul(
                        ps[:, q, :], hT8[:, 2 * kp:2 * kp + 2, :],
                        w[:, 2 * kp:2 * kp + 2,
                          q * D + g * GS:q * D + (g + 1) * GS],
                        start=(kp == 0 and q in (0, 2)),
                        stop=False,
                        perf_mode=PM.DoubleRow,
                        skip_group_check=True)

        def transpose_quad(hs_pair, hT, half):
            # hs_pair: two [128, 256] h slices -> hT[:, 4*half:4*half+4, :]
            trq = trpool.tile([128, 512], BF16, tag="tr")
            for j in range(2):
                nc.tensor.transpose(trq[:, 128 * j:128 * (j + 1)],
                                    hs_pair[0][:, 128 * j:128 * (j + 1)], idbf)
                nc.tensor.transpose(trq[:, 256 + 128 * j:256 + 128 * (j + 1)],
                                    hs_pair[1][:, 128 * j:128 * (j + 1)], idbf)
            nc.scalar.copy(out=hT[:, 4 * half:4 * half + 4, :], in_=trq)

        for t in range(nsteps):
            # --- layer 0: gates0 = x0_proj + h0 @ W_hh0^T (fp8 DoubleRow) ---
            hs0 = []
            for g in range(NG):
                ps = gpool.tile([128, 4, GS], F32, tag="g")
                hh_matmuls(ps, h0T8, whh0, g)
                # close the accumulation for the sim on the last writes
                hs = hst.tile([128, GS], BF16, tag="h")
                hs0.append(hs)
                cell_math(g, ps, c0, x0sb[:, g], hs)

            # --- layer 1: h0 transpose quads hoisted as early as their cell
            # math allows, pipelined in halves across the hh window ---
            hf = hfp.tile([128, KC, 128], F32, tag="hf")
            ps1 = []
            # g0/g1 cell math finished during L0 g2/g3 matmuls
            transpose_quad(hs0[0:2], h0T, 0)
            ps = gpool.tile([128, 4, GS], F32, tag="g")
            ps1.append(ps)
            hh_matmuls(ps, h1T8, whh1, 0)
            nc.scalar.copy(out=hf[:, 0:4, :], in_=h0T[:, 0:4, :])
            nc.vector.tensor_scalar_mul(h0T8[:, 0:4, :], hf[:, 0:4, :], HS)
            ps = gpool.tile([128, 4, GS], F32, tag="g")
            ps1.append(ps)
            hh_matmuls(ps, h1T8, whh1, 1)
            # second half (g3's cell math finishes ~2.5us after L0's matmuls)
            transpose_quad(hs0[2:4], h0T, 1)
            nc.scalar.copy(out=hf[:, 4:8, :], in_=h0T[:, 4:8, :])
            nc.vector.tensor_scalar_mul(h0T8[:, 4:8, :], hf[:, 4:8, :], HS)
            for g in range(2, 4):
                ps = gpool.tile([128, 4, GS], F32, tag="g")
                ps1.append(ps)
                hh_matmuls(ps, h1T8, whh1, g)

            h1T_new = h1Tp.tile([128, KC, 128], BF16, tag="h1T")
            h1T8_new = h1T8p.tile([128, KC, 128], FP8, tag="h1T8")
            hs1 = []
            for g in range(NG):
                ps = ps1[g]
                # i,f,o gate columns: fp8 DoubleRow against h0T8
                for kp in range(KC // 2):
                    for q in range(3):
                        nc.tensor.matmul(
                            ps[:, q, :], h0T8[:, 2 * kp:2 * kp + 2, :],
                            wih1_8[:, 2 * kp:2 * kp + 2,
                                   q * D + g * GS:q * D + (g + 1) * GS],
                            start=False,
                            stop=(kp == KC // 2 - 1),
                            perf_mode=PM.DoubleRow,
                            skip_group_check=True)
                # g gate columns: bf16 against h0T
                for k in range(KC):
                    nc.tensor.matmul(
                        ps[:, 3, :], h0T[:, k, :],
                        wih1_g[:, k, g * GS:(g + 1) * GS],
                        start=False,
                        stop=(k == KC - 1),
                        skip_group_check=True)
                hs = hst.tile([128, GS], BF16, tag="h")
                hs1.append(hs)
                cell_math(g, ps, c1, b1rep[:, 4 * g:4 * g + 4, :], hs)
                if g == 1:
                    transpose_quad(hs1[0:2], h1T_new, 0)
            transpose_quad(hs1[2:4], h1T_new, 1)

            # --- inline final linear: y_t = h1_t @ W_lin^T + b_lin ---
            ys = ysp.tile([128, D], F32, tag="ys")
            for cch in range(4):
                yp = ypool.tile([128, 256], F32, tag="y")
                for k in range(KC):
                    nc.tensor.matmul(
                        yp, h1T_new[:, k, :],
                        wlin[:, k, cch * 256:(cch + 1) * 256],
                        start=(k == 0), stop=(k == KC - 1),
                        skip_group_check=True)
                nc.vector.tensor_add(ys[:, cch * 256:(cch + 1) * 256], yp,
                                     blin[:, cch * 256:(cch + 1) * 256])
            nc.sync.dma_start(out=d_out[:, t, :], in_=ys)

            # h1 fp8 copy for next step's L1 hh (not latency-critical: runs
            # on Act/DVE while the PE does y and the next step's L0)
            hf1 = hfp.tile([128, KC, 128], F32, tag="hf")
            nc.scalar.copy(out=hf1, in_=h1T_new)
            nc.vector.tensor_scalar_mul(h1T8_new, hf1, HS)

            if dbg and t == 0:
                nc.sync.dma_start(out=dbg["h0T1"], in_=h0T)
                nc.sync.dma_start(out=dbg["c01"], in_=c0)
                nc.sync.dma_start(out=dbg["h1T1"], in_=h1T_new)
                nc.sync.dma_start(out=dbg["c11"], in_=c1)
            h1T8 = h1T8_new

    post.release()
    h1T8p.release()
    h1Tp.release()
    life.release()


_CACHE = {}


def _get_module(nsteps=BAR):
    if nsteps not in _CACHE:
        _CACHE[nsteps] = build_module(nsteps)
    return _CACHE[nsteps]


def prep_inputs(z, bn_gamma, bn_beta, W_ih0, W_hh0, b_ih0, b_hh0,
                W_ih1, W_hh1, b_ih1, b_hh1, W_lin, b_lin):
    z = np.asarray(z, np.float32)
    zT = np.ascontiguousarray(z.T)
    wih1p = _gate_perm(np.asarray(W_ih1, np.float32).T)  # [D, 4, D] i,f,o,g
    common = {
        "zT": _np_bf16(zT),
        "gammaT": np.ascontiguousarray(
            np.asarray(bn_gamma, np.float32).reshape(KC, 128).T),
        "betaT": np.ascontiguousarray(
            np.asarray(bn_beta, np.float32).reshape(KC, 128).T),
        # x2048 prescale on bf16 so gate contributions match the fp8 (x32*x64)
        # PSUM scale; activations divide it back out.
        "wt_ih0": _np_bf16(_gate_perm(np.asarray(W_ih0, np.float32).T)
                           .reshape(D, H4) * GSC),
        "wt_hh0": _np_fp8(_gate_perm(np.asarray(W_hh0, np.float32).T)
                          .reshape(D, H4) * WS),
        "wt_ih1_8": _np_fp8(wih1p[:, 0:3, :].reshape(D, 3 * D) * WS),
        "wt_ih1_g": _np_bf16(wih1p[:, 3, :] * GSC),
        "wt_hh1": _np_fp8(_gate_perm(np.asarray(W_hh1, np.float32).T)
                          .reshape(D, H4) * WS),
        "wt_lin": _np_bf16(np.asarray(W_lin, np.float32).T),
        "b0r": _np_bf16(GSC * _gate_bias(np.asarray(b_ih0, np.float32)
                                         + np.asarray(b_hh0, np.float32))
                        .reshape(H4)),
        "b1r": _np_bf16(GSC * _gate_bias(np.asarray(b_ih1, np.float32)
                                         + np.asarray(b_hh1, np.float32))
                        .reshape(H4)),
        "b_lin": np.asarray(b_lin, np.float32),
    }
    in_maps = []
    for c in range(NCORES):
        m = dict(common)
        m["zTs"] = np.ascontiguousarray(zT[:, c * BS:(c + 1) * BS])
        in_maps.append(m)
    return in_maps


def kernel(**inputs):
    nc = _get_module()
    in_maps = prep_inputs(**inputs)
    res = bass_utils.run_bass_kernel_spmd(nc, in_maps, core_ids=list(range(NCORES)))
    out = np.concatenate([res.results[c]["out"] for c in range(NCORES)], axis=0)
    return out.astype(np.float32)


# revision 35
# speedup vs baseline: 1.0504x; 1.0504x over previous
"""Trainium2 Bass kernel: BatchNorm -> 2-layer LSTM (32 steps, constant layer-0
input) -> Linear, data-parallel over batch across 8 NeuronCores.

v2 layout strategy (per core, batch shard = 128 rows):
  - All gate matmuls are out[b, j] = lhsT.T @ rhs with lhsT = h^T chunks
    (stationary) and rhs = W^T chunks (moving), fp32 PSUM accumulation.
  - fp8 DoubleRow (K=256/pass) for W_hh0, W_hh1 and the i,f,o gate columns of
    W_ih1; the g-gate column block of W_ih1 stays bf16 (tanh path feeds c
    linearly and dominates the error budget).
  - Biases enter via DVE adds against DMA-replicated bias rows (no K=1 bias
    matmuls on the PE).
  - The final linear y_t = h1_t @ W_lin^T + b_lin is computed inline each step
    (N=256 chunks) instead of a deferred phase; no h1 HBM roundtrip.
  - Init: BN stats batched across all 8 feature chunks, W_ih0 streamed once,
    weight DMAs ordered by first-use time.
"""

import os
import sys

sys.path.insert(0, "/opt/trn_rl_repo")

import numpy as np
import ml_dtypes

import concourse.bass as bass
import concourse.bacc as bacc
import concourse.tile as tile
import concourse.mybir as mybir
from concourse import bass_utils
from concourse.masks import make_identity

BF16 = mybir.dt.bfloat16
FP8 = mybir.dt.float8e4
F32 = mybir.dt.float32
AF = mybir.ActivationFunctionType
ALU = mybir.AluOpType
PM = mybir.MatmulPerfMode

# fp8 scaling: recurrent weights x64, h state x32 -> gate PSUM lands x2048.
# bf16 weights/biases are pre-scaled x2048 on host so all gate contributions
# agree; activations fold in 1/2048 via their scale argument.
WS = 64.0
HS = 32.0
GSC = WS * HS
INV_GSC = 1.0 / GSC

B = 1024          # batch
D = 1024          # hidden = input size
H4 = 4 * D        # gate width
BAR = int(os.environ.get("KERNEL_NSTEPS", "32"))
NCORES = int(os.environ.get("KERNEL_NCORES", "8"))
BS = B // 8       # batch shard per core
EPS = 1e-5
KC = D // 128     # contraction chunks (8)
NG = 4            # gate column groups per layer
GS = D // NG      # group size in hidden cols (256)


def _np_bf16(a):
    return np.ascontiguousarray(a).astype(ml_dtypes.bfloat16)


def _np_fp8(a):
    return np.ascontiguousarray(a).astype(ml_dtypes.float8_e4m3)


GPERM = [0, 1, 3, 2]  # device gate order i, f, o, g (one wide sigmoid over 0:3)


def _gate_perm(wT):
    # wT is W.T with 4 gate blocks of D columns; reorder blocks to GPERM
    return wT.reshape(D, 4, D)[:, GPERM, :]


def _gate_bias(b):
    # [4H] -> [16, 256] tile layout, group-major, gate order GPERM
    return (np.asarray(b, np.float32).reshape(4, 4, 256)[GPERM]
            .transpose(1, 0, 2).reshape(16, 256))


def build_module(nsteps=BAR):
    nc = bacc.Bacc(
        "TRN2",
        target_bir_lowering=False,
        debug=False,
        enable_asserts=False,
        num_devices=NCORES,
        dynamic_dma_scratch_size=512,
    )

    # ---- DRAM I/O -------------------------------------------------------
    d_zT = nc.dram_tensor("zT", [D, B], BF16, kind="ExternalInput").ap()
    d_zTs = nc.dram_tensor("zTs", [D, BS], F32, kind="ExternalInput").ap()
    d_gT = nc.dram_tensor("gammaT", [128, KC], F32, kind="ExternalInput").ap()
    d_bT = nc.dram_tensor("betaT", [128, KC], F32, kind="ExternalInput").ap()
    d_wih0 = nc.dram_tensor("wt_ih0", [D, H4], BF16, kind="ExternalInput").ap()
    d_whh0 = nc.dram_tensor("wt_hh0", [D, H4], FP8, kind="ExternalInput").ap()
    d_wih1_8 = nc.dram_tensor("wt_ih1_8", [D, 3 * D], FP8,
                              kind="ExternalInput").ap()
    d_wih1_g = nc.dram_tensor("wt_ih1_g", [D, D], BF16,
                              kind="ExternalInput").ap()
    d_whh1 = nc.dram_tensor("wt_hh1", [D, H4], FP8, kind="ExternalInput").ap()
    d_wlin = nc.dram_tensor("wt_lin", [D, D], BF16, kind="ExternalInput").ap()
    d_b0 = nc.dram_tensor("b0r", [H4], BF16, kind="ExternalInput").ap()
    d_b1 = nc.dram_tensor("b1r", [H4], BF16, kind="ExternalInput").ap()
    d_blin = nc.dram_tensor("b_lin", [D], F32, kind="ExternalInput").ap()
    d_out = nc.dram_tensor("out", [BS, nsteps, D], F32, kind="ExternalOutput").ap()
    dbg = {}
    if os.environ.get("KERNEL_DEBUG"):
        dbg["znT"] = nc.dram_tensor("dbg_znT", [128, KC, 128], F32,
                                    kind="ExternalOutput").ap()
        dbg["c0i"] = nc.dram_tensor("dbg_c0i", [128, D], F32,
                                    kind="ExternalOutput").ap()
        dbg["x0"] = nc.dram_tensor("dbg_x0", [4, 128, 4, 256], BF16,
                                   kind="ExternalOutput").ap()
        dbg["h0T1"] = nc.dram_tensor("dbg_h0T1", [128, KC, 128], BF16,
                                     kind="ExternalOutput").ap()
        dbg["c01"] = nc.dram_tensor("dbg_c01", [128, D], F32,
                                    kind="ExternalOutput").ap()
        dbg["h1T1"] = nc.dram_tensor("dbg_h1T1", [128, KC, 128], BF16,
                                     kind="ExternalOutput").ap()
        dbg["c11"] = nc.dram_tensor("dbg_c11", [128, D], F32,
                                    kind="ExternalOutput").ap()

    with tile.TileContext(nc) as tc:
        build_body(nc, tc, nsteps,
                   d_zT, d_zTs, d_gT, d_bT,
                   d_wih0, d_whh0, d_wih1_8, d_wih1_g, d_whh1, d_wlin,
                   d_b0, d_b1, d_blin, d_out, dbg)
    nc.compile()
    return nc


def build_body(nc, tc, nsteps, d_zT, d_zTs, d_gT, d_bT,
               d_wih0, d_whh0, d_wih1_8, d_wih1_g, d_whh1, d_wlin,
               d_b0, d_b1, d_blin, d_out, dbg):
    # ---- whole-life SBUF ------------------------------------------------
    life = tc.alloc_tile_pool(name="life", bufs=1)
    whh0 = life.tile([128, KC, H4], FP8, tag="whh0")
    whh1 = life.tile([128, KC, H4], FP8, tag="whh1")
    wih1_8 = life.tile([128, KC, 3 * D], FP8, tag="wih1_8")
    c0 = life.tile([128, D], F32, tag="c0")
    c1 = life.tile([128, D], F32, tag="c1")
    h0T = life.tile([128, KC, 128], BF16, tag="h0T")
    h0T8 = life.tile([128, KC, 128], FP8, tag="h0T8")
    x0sb = life.tile([128, NG, 4, GS], BF16, tag="x0sb")
    idbf = life.tile([128, 128], BF16, tag="idbf")

    h1Tp = tc.alloc_tile_pool(name="h1Tp", bufs=2)
    h1T8p = tc.alloc_tile_pool(name="h1T8p", bufs=2)
    h1T8_init = h1T8p.tile([128, KC, 128], FP8, tag="h1T8", name="h1T8_init")

    make_identity(nc, idbf)

    # ---- INIT phase: BN stats + zn^T + c0/c1 + x0_proj ------------------
    with tc.tile_pool(name="initp", bufs=8) as initp, \
         tc.tile_pool(name="wkp", bufs=3) as wkp, \
         tc.tile_pool(name="ismall", bufs=1) as ismall, \
         tc.tile_pool(name="ipsum", bufs=4, space="PSUM") as ipsum:

        # -- DMA issue order: stats inputs, then weights by first use.
        # Big weights go on the Sync queue; the wk stream and lower-priority
        # loads go on the GpSimd queue so neither blocks the other. --
        zs_all = ismall.tile([128, KC, BS], F32, tag="zs")
        zs_b = bass.AP(tensor=d_zTs.tensor, offset=d_zTs.offset,
                       ap=[[BS, 128], [128 * BS, KC], [1, BS]])
        nc.sync.dma_start(out=zs_all, in_=zs_b)
        gT = ismall.tile([128, KC], F32, tag="gT")
        nc.sync.dma_start(out=gT, in_=d_gT)
        bT = ismall.tile([128, KC], F32, tag="bT")
        nc.sync.dma_start(out=bT, in_=d_bT)
        zts = []
        for k in range(KC):
            zt = initp.tile([128, B], BF16, tag="zt", name=f"zt{k}")
            nc.sync.dma_start(out=zt, in_=d_zT[k * 128:(k + 1) * 128, :])
            zts.append(zt)
        for k in range(KC):
            nc.sync.dma_start(out=whh0[:, k, :],
                              in_=d_whh0[k * 128:(k + 1) * 128, :])
        b0rep = ismall.tile([128, 16, 256], BF16, tag="b0rep")
        b0_b = bass.AP(tensor=d_b0.tensor, offset=d_b0.offset,
                       ap=[[0, 128], [1, H4]])
        nc.sync.dma_start(out=b0rep, in_=b0_b)
        # whh1/wih1_8 next on the Sync queue: needed from step 0's layer 1
        for k in range(KC):
            nc.sync.dma_start(out=whh1[:, k, :],
                              in_=d_whh1[k * 128:(k + 1) * 128, :])
        for k in range(KC):
            nc.sync.dma_start(out=wih1_8[:, k, :],
                              in_=d_wih1_8[k * 128:(k + 1) * 128, :])

        eps_t = ismall.tile([128, 1], F32, tag="eps")
        nc.vector.memset(eps_t, EPS)
        znf = ismall.tile([128, KC, 128], F32, tag="znf")
        idf32 = ismall.tile([128, 128], F32, tag="idf32")
        make_identity(nc, idf32)

        st = ismall.tile([128, KC, 2, 6], F32, tag="st")
        mv = ismall.tile([128, KC, 2], F32, tag="mv")
        sd8 = ismall.tile([128, KC], F32, tag="sd8")
        rs8 = ismall.tile([128, KC], F32, tag="rs8")
        sc8 = ismall.tile([128, KC], F32, tag="sc8")

        for k in range(KC):
            nc.vector.bn_stats(out=st[:, k, 0, :], in_=zts[k][:, 0:512])
            nc.vector.bn_stats(out=st[:, k, 1, :], in_=zts[k][:, 512:1024])
            nc.vector.bn_aggr(out=mv[:, k, :], in_=st[:, k])
        nc.scalar.activation(out=sd8, in_=mv[:, :, 1:2], func=AF.Sqrt,
                             bias=eps_t)
        nc.vector.reciprocal(out=rs8, in_=sd8)
        nc.vector.tensor_mul(sc8, gT, rs8)

        for k in range(KC):
            # zn^T chunk (fp32): (z - mean) * scale + beta
            nc.vector.tensor_scalar(
                out=znf[:, k, :], in0=zs_all[:, k, :],
                scalar1=mv[:, k, 0:1], scalar2=sc8[:, k:k + 1],
                op0=ALU.subtract, op1=ALU.mult)
            nc.vector.tensor_scalar_add(znf[:, k, :], znf[:, k, :],
                                        bT[:, k:k + 1])
            # bf16 copy for matmul lhsT (h0 initial state) + fp8 x32 copies
            # (fp8 must be produced from fp32 — bf16->fp8 converts are broken)
            nc.scalar.copy(out=h0T[:, k, :], in_=znf[:, k, :])
            nc.vector.tensor_scalar_mul(h0T8[:, k, :], znf[:, k, :], HS)
            nc.vector.tensor_scalar_mul(h1T8_init[:, k, :], znf[:, k, :], HS)

        if dbg:
            nc.sync.dma_start(out=dbg["znT"], in_=znf)

        # x0_proj = zn @ W_ih0^T + (b_ih0 + b_hh0), group-major bf16.
        # W_ih0 streamed once; all 4 group PSUMs live (8 banks).
        psg = [ipsum.tile([128, 4, GS], F32, tag="ips", name=f"ips{g}")
               for g in range(NG)]
        for k in range(KC):
            wk = wkp.tile([128, H4], BF16, tag="wi0")
            nc.sync.dma_start(out=wk, in_=d_wih0[k * 128:(k + 1) * 128, :])
            for g in range(NG):
                for q in range(4):
                    nc.tensor.matmul(
                        psg[g][:, q, :], h0T[:, k, :],
                        wk[:, q * D + g * GS:q * D + (g + 1) * GS],
                        start=(k == 0 and q in (0, 2)),
                        stop=(k == KC - 1),
                        skip_group_check=True)
        for g in range(NG):
            nc.vector.tensor_add(x0sb[:, g], psg[g], b0rep[:, 4 * g:4 * g + 4, :])
            if dbg:
                nc.sync.dma_start(out=dbg["x0"][g], in_=x0sb[:, g])

        # c0 = c1 = zn in [b, d] layout via PE transpose of fp32 zn^T
        # (after x0proj so the transposes don't stall the PE early)
        for k in range(KC):
            pt = ipsum.tile([128, 4, GS], F32, tag="ips", name=f"tpz{k}")
            nc.tensor.transpose(pt[:, 0, 0:128], znf[:, k, :], idf32)
            nc.scalar.copy(out=c0[:, k * 128:(k + 1) * 128], in_=pt[:, 0, 0:128])
        nc.vector.tensor_copy(out=c1, in_=c0)
        if dbg:
            nc.sync.dma_start(out=dbg["c0i"], in_=c0)

    # ---- post-init loads (reuse init SBUF space) ------------------------
    post = tc.alloc_tile_pool(name="post", bufs=1)
    b1rep = post.tile([128, 16, 256], BF16, tag="b1rep")
    b1_b = bass.AP(tensor=d_b1.tensor, offset=d_b1.offset,
                   ap=[[0, 128], [1, H4]])
    nc.sync.dma_start(out=b1rep, in_=b1_b)
    blin = post.tile([128, D], F32, tag="blin")
    blin_b = bass.AP(tensor=d_blin.tensor, offset=d_blin.offset,
                     ap=[[0, 128], [1, D]])
    nc.sync.dma_start(out=blin, in_=blin_b)
    wih1_g = post.tile([128, KC, D], BF16, tag="wih1_g")
    for k in range(KC):
        nc.sync.dma_start(out=wih1_g[:, k, :],
                          in_=d_wih1_g[k * 128:(k + 1) * 128, :])
    wlin = post.tile([128, KC, D], BF16, tag="wlin")
    for k in range(KC):
        nc.sync.dma_start(out=wlin[:, k, :],
                          in_=d_wlin[k * 128:(k + 1) * 128, :])

    # ---- recurrent loop --------------------------------------------------
    with tc.tile_pool(name="gates", bufs=3, space="PSUM") as gpool, \
         tc.tile_pool(name="trp", bufs=1, space="PSUM") as trpool, \
         tc.tile_pool(name="yps", bufs=1, space="PSUM") as ypool, \
         tc.tile_pool(name="tmp", bufs=3) as tmp, \
         tc.tile_pool(name="hfp", bufs=2) as hfp, \
         tc.tile_pool(name="hst", bufs=4) as hst, \
         tc.tile_pool(name="ysp", bufs=2) as ysp:

        h1T8 = h1T8_init

        def cell_math(g, ps, c, x0_slice, hs):
            # gates: ps[:, 0..3, :] = i, f, o, g (pre-activation x GSC, fp32)
            # psum + sbuf -> sbuf: releases the PSUM tile after one DVE op
            src = tmp.tile([128, 4, GS], F32, tag="sum")
            nc.vector.tensor_add(src, ps, x0_slice)
            sg = tmp.tile([128, 3, GS], BF16, tag="sg")
            nc.scalar.activation(out=sg, in_=src[:, 0:3, :], func=AF.Sigmoid,
                                 scale=INV_GSC)
            tg = tmp.tile([128, GS], BF16, tag="tg")
            nc.scalar.activation(out=tg, in_=src[:, 3, :], func=AF.Tanh,
                                 scale=INV_GSC)
            csl = c[:, g * GS:(g + 1) * GS]
            nc.vector.tensor_mul(csl, csl, sg[:, 1, :])          # c *= sig(f)
            tp = tmp.tile([128, GS], F32, tag="tp")
            nc.vector.tensor_mul(tp, sg[:, 0, :], tg)            # sig(i)*tanh(g)
            nc.vector.tensor_add(csl, csl, tp)
            tc2 = tmp.tile([128, GS], BF16, tag="tc2")
            nc.scalar.activation(out=tc2, in_=csl, func=AF.Tanh)
            nc.vector.tensor_mul(hs, sg[:, 2, :], tc2)           # h = sig(o)*tanh(c)

        def hh_matmuls(ps, hT8, w, g):
            for kp in range(KC // 2):
                for q in range(4):
                    nc.tensor.matmul(
                        ps[:, q, :], hT8[:, 2 * kp:2 * kp + 2, :],
                        w[:, 2 * kp:2 * kp + 2,
                          q * D + g * GS:q * D + (g + 1) * GS],
                        start=(kp == 0 and q in (0, 2)),
                        stop=False,
                        perf_mode=PM.DoubleRow,
                        skip_group_check=True)

        def transpose_quad(hs_pair, hT, half):
            # hs_pair: two [128, 256] h slices -> hT[:, 4*half:4*half+4, :]
            trq = trpool.tile([128, 512], BF16, tag="tr")
            for j in range(2):
                nc.tensor.transpose(trq[:, 128 * j:128 * (j + 1)],
                                    hs_pair[0][:, 128 * j:128 * (j + 1)], idbf)
                nc.tensor.transpose(trq[:, 256 + 128 * j:256 + 128 * (j + 1)],
                                    hs_pair[1][:, 128 * j:128 * (j + 1)], idbf)
            nc.scalar.copy(out=hT[:, 4 * half:4 * half + 4, :], in_=trq)

        for t in range(nsteps):
            # --- layer 0: gates0 = x0_proj + h0 @ W_hh0^T (fp8 DoubleRow) ---
            hs0 = []
            for g in range(NG):
                ps = gpool.tile([128, 4, GS], F32, tag="g")
                hh_matmuls(ps, h0T8, whh0, g)
                # close the accumulation for the sim on the last writes
                hs = hst.tile([128, GS], BF16, tag="h")
                hs0.append(hs)
                cell_math(g, ps, c0, x0sb[:, g], hs)

            # --- layer 1: hh first (uses h1T8 from t-1), h0 transposes
            # pipelined in halves across the hh window, then ih ---
            hf = hfp.tile([128, KC, 128], F32, tag="hf")
            ps1 = []
            for g in range(2):
                ps = gpool.tile([128, 4, GS], F32, tag="g")
                ps1.append(ps)
                hh_matmuls(ps, h1T8, whh1, g)
            # first half of h0: transposes, upcast, fp8 convert (pipelined
            # so the h0T8 chain isn't serialized behind g3's cell math)
            transpose_quad(hs0[0:2], h0T, 0)
            nc.scalar.copy(out=hf[:, 0:4, :], in_=h0T[:, 0:4, :])
            nc.vector.tensor_scalar_mul(h0T8[:, 0:4, :], hf[:, 0:4, :], HS)
            for g in range(2, 4):
                ps = gpool.tile([128, 4, GS], F32, tag="g")
                ps1.append(ps)
                hh_matmuls(ps, h1T8, whh1, g)
            # second half (L0 g3 cell math has finished by now)
            transpose_quad(hs0[2:4], h0T, 1)
            nc.scalar.copy(out=hf[:, 4:8, :], in_=h0T[:, 4:8, :])
            nc.vector.tensor_scalar_mul(h0T8[:, 4:8, :], hf[:, 4:8, :], HS)

            h1T_new = h1Tp.tile([128, KC, 128], BF16, tag="h1T")
            h1T8_new = h1T8p.tile([128, KC, 128], FP8, tag="h1T8")
            hs1 = []
            for g in range(NG):
                ps = ps1[g]
                # i,f,o gate columns: fp8 DoubleRow against h0T8
                for kp in range(KC // 2):
                    for q in range(3):
                        nc.tensor.matmul(
                            ps[:, q, :], h0T8[:, 2 * kp:2 * kp + 2, :],
                            wih1_8[:, 2 * kp:2 * kp + 2,
                                   q * D + g * GS:q * D + (g + 1) * GS],
                            start=False,
                            stop=(kp == KC // 2 - 1),
                            perf_mode=PM.DoubleRow,
                            skip_group_check=True)
                # g gate columns: bf16 against h0T
                for k in range(KC):
                    nc.tensor.matmul(
                        ps[:, 3, :], h0T[:, k, :],
                        wih1_g[:, k, g * GS:(g + 1) * GS],
                        start=False,
                        stop=(k == KC - 1),
                        skip_group_check=True)
                hs = hst.tile([128, GS], BF16, tag="h")
                hs1.append(hs)
                cell_math(g, ps, c1, b1rep[:, 4 * g:4 * g + 4, :], hs)
                if g == 1:
                    transpose_quad(hs1[0:2], h1T_new, 0)
            transpose_quad(hs1[2:4], h1T_new, 1)

            # --- inline final linear: y_t = h1_t @ W_lin^T + b_lin ---
            ys = ysp.tile([128, D], F32, tag="ys")
            for cch in range(4):
                yp = ypool.tile([128, 256], F32, tag="y")
                for k in range(KC):
                    nc.tensor.matmul(
                        yp, h1T_new[:, k, :],
                        wlin[:, k, cch * 256:(cch + 1) * 256],
                        start=(k == 0), stop=(k == KC - 1),
                        skip_group_check=True)
                nc.vector.tensor_add(ys[:, cch * 256:(cch + 1) * 256], yp,
                                     blin[:, cch * 256:(cch + 1) * 256])
            nc.sync.dma_start(out=d_out[:, t, :], in_=ys)

            # h1 fp8 copy for next step's L1 hh (not latency-critical: runs
            # on Act/DVE while the PE does y and the next step's L0)
            hf1 = hfp.tile([128, KC, 128], F32, tag="hf")
            nc.scalar.copy(out=hf1, in_=h1T_new)
            nc.vector.tensor_scalar_mul(h1T8_new, hf1, HS)

            if dbg and t == 0:
                nc.sync.dma_start(out=dbg["h0T1"], in_=h0T)
                nc.sync.dma_start(out=dbg["c01"], in_=c0)
                nc.sync.dma_start(out=dbg["h1T1"], in_=h1T_new)
                nc.sync.dma_start(out=dbg["c11"], in_=c1)
            h1T8 = h1T8_new

    post.release()
    h1T8p.release()
    h1Tp.release()
    life.release()


_CACHE = {}


def _get_module(nsteps=BAR):
    if nsteps not in _CACHE:
        _CACHE[nsteps] = build_module(nsteps)
    return _CACHE[nsteps]


def prep_inputs(z, bn_gamma, bn_beta, W_ih0, W_hh0, b_ih0, b_hh0,
                W_ih1, W_hh1, b_ih1, b_hh1, W_lin, b_lin):
    z = np.asarray(z, np.float32)
    zT = np.ascontiguousarray(z.T)
    wih1p = _gate_perm(np.asarray(W_ih1, np.float32).T)  # [D, 4, D] i,f,o,g
    common = {
        "zT": _np_bf16(zT),
        "gammaT": np.ascontiguousarray(
            np.asarray(bn_gamma, np.float32).reshape(KC, 128).T),
        "betaT": np.ascontiguousarray(
            np.asarray(bn_beta, np.float32).reshape(KC, 128).T),
        # x2048 prescale on bf16 so gate contributions match the fp8 (x32*x64)
        # PSUM scale; activations divide it back out.
        "wt_ih0": _np_bf16(_gate_perm(np.asarray(W_ih0, np.float32).T)
                           .reshape(D, H4) * GSC),
        "wt_hh0": _np_fp8(_gate_perm(np.asarray(W_hh0, np.float32).T)
                          .reshape(D, H4) * WS),
        "wt_ih1_8": _np_fp8(wih1p[:, 0:3, :].reshape(D, 3 * D) * WS),
        "wt_ih1_g": _np_bf16(wih1p[:, 3, :] * GSC),
        "wt_hh1": _np_fp8(_gate_perm(np.asarray(W_hh1, np.float32).T)
                          .reshape(D, H4) * WS),
        "wt_lin": _np_bf16(np.asarray(W_lin, np.float32).T),
        "b0r": _np_bf16(GSC * _gate_bias(np.asarray(b_ih0, np.float32)
                                         + np.asarray(b_hh0, np.float32))
                        .reshape(H4)),
        "b1r": _np_bf16(GSC * _gate_bias(np.asarray(b_ih1, np.float32)
                                         + np.asarray(b_hh1, np.float32))
                        .reshape(H4)),
        "b_lin": np.asarray(b_lin, np.float32),
    }
    in_maps = []
    for c in range(NCORES):
        m = dict(common)
        m["zTs"] = np.ascontiguousarray(zT[:, c * BS:(c + 1) * BS])
        in_maps.append(m)
    return in_maps


def kernel(**inputs):
    nc = _get_module()
    in_maps = prep_inputs(**inputs)
    res = bass_utils.run_bass_kernel_spmd(nc, in_maps, core_ids=list(range(NCORES)))
    out = np.concatenate([res.results[c]["out"] for c in range(NCORES)], axis=0)
    return out.astype(np.float32)


# revision 39
# speedup vs baseline: 1.0699x; 1.0185x over previous
"""Trainium2 Bass kernel: BatchNorm -> 2-layer LSTM (32 steps, constant layer-0
input) -> Linear, data-parallel over batch across 8 NeuronCores.

v2 layout strategy (per core, batch shard = 128 rows):
  - All gate matmuls are out[b, j] = lhsT.T @ rhs with lhsT = h^T chunks
    (stationary) and rhs = W^T chunks (moving), fp32 PSUM accumulation.
  - fp8 DoubleRow (K=256/pass) for W_hh0, W_hh1 and the i,f,o gate columns of
    W_ih1; the g-gate column block of W_ih1 stays bf16 (tanh path feeds c
    linearly and dominates the error budget).
  - Biases enter via DVE adds against DMA-replicated bias rows (no K=1 bias
    matmuls on the PE).
  - The final linear y_t = h1_t @ W_lin^T + b_lin is computed inline each step
    (N=256 chunks) instead of a deferred phase; no h1 HBM roundtrip.
  - Init: BN stats batched across all 8 feature chunks, W_ih0 streamed once,
    weight DMAs ordered by first-use time.
"""

import os
import sys

sys.path.insert(0, "/opt/trn_rl_repo")

import numpy as np
import ml_dtypes

import concourse.bass as bass
import concourse.bacc as bacc
import concourse.tile as tile
import concourse.mybir as mybir
from concourse import bass_utils
from concourse.masks import make_identity

BF16 = mybir.dt.bfloat16
FP8 = mybir.dt.float8e4
F32 = mybir.dt.float32
AF = mybir.ActivationFunctionType
ALU = mybir.AluOpType
PM = mybir.MatmulPerfMode

# fp8 scaling: recurrent weights x64, h state x32 -> gate PSUM lands x2048.
# bf16 weights/biases are pre-scaled x2048 on host so all gate contributions
# agree; activations fold in 1/2048 via their scale argument.
WS = 64.0
HS = 32.0
GSC = WS * HS
INV_GSC = 1.0 / GSC

B = 1024          # batch
D = 1024          # hidden = input size
H4 = 4 * D        # gate width
BAR = int(os.environ.get("KERNEL_NSTEPS", "32"))
NCORES = int(os.environ.get("KERNEL_NCORES", "8"))
BS = B // 8       # batch shard per core
EPS = 1e-5
KC = D // 128     # contraction chunks (8)
NG = 4            # gate column groups per layer
GS = D // NG      # group size in hidden cols (256)


def _np_bf16(a):
    return np.ascontiguousarray(a).astype(ml_dtypes.bfloat16)


def _np_fp8(a):
    return np.ascontiguousarray(a).astype(ml_dtypes.float8_e4m3)


GPERM = [0, 1, 3, 2]  # device gate order i, f, o, g (one wide sigmoid over 0:3)


def _gate_perm(wT):
    # wT is W.T with 4 gate blocks of D columns; reorder blocks to GPERM
    return wT.reshape(D, 4, D)[:, GPERM, :]


def _gate_bias(b):
    # [4H] -> [16, 256] tile layout, group-major, gate order GPERM
    return (np.asarray(b, np.float32).reshape(4, 4, 256)[GPERM]
            .transpose(1, 0, 2).reshape(16, 256))


def build_module(nsteps=BAR):
    nc = bacc.Bacc(
        "TRN2",
        target_bir_lowering=False,
        debug=False,
        enable_asserts=False,
        num_devices=NCORES,
        dynamic_dma_scratch_size=512,
    )

    # ---- DRAM I/O -------------------------------------------------------
    d_zT = nc.dram_tensor("zT", [D, B], BF16, kind="ExternalInput").ap()
    d_zTs = nc.dram_tensor("zTs", [D, BS], F32, kind="ExternalInput").ap()
    d_gT = nc.dram_tensor("gammaT", [128, KC], F32, kind="ExternalInput").ap()
    d_bT = nc.dram_tensor("betaT", [128, KC], F32, kind="ExternalInput").ap()
    d_wih0 = nc.dram_tensor("wt_ih0", [D, H4], BF16, kind="ExternalInput").ap()
    d_whh0 = nc.dram_tensor("wt_hh0", [D, H4], FP8, kind="ExternalInput").ap()
    d_wih1_8 = nc.dram_tensor("wt_ih1_8", [D, 3 * D], FP8,
                              kind="ExternalInput").ap()
    d_wih1_g = nc.dram_tensor("wt_ih1_g", [D, D], BF16,
                              kind="ExternalInput").ap()
    d_whh1 = nc.dram_tensor("wt_hh1", [D, H4], FP8, kind="ExternalInput").ap()
    d_wlin = nc.dram_tensor("wt_lin", [D, D], BF16, kind="ExternalInput").ap()
    d_b0 = nc.dram_tensor("b0r", [H4], BF16, kind="ExternalInput").ap()
    d_b1 = nc.dram_tensor("b1r", [H4], BF16, kind="ExternalInput").ap()
    d_blin = nc.dram_tensor("b_lin", [D], F32, kind="ExternalInput").ap()
    d_out = nc.dram_tensor("out", [BS, nsteps, D], F32, kind="ExternalOutput").ap()
    dbg = {}
    if os.environ.get("KERNEL_DEBUG"):
        dbg["znT"] = nc.dram_tensor("dbg_znT", [128, KC, 128], F32,
                                    kind="ExternalOutput").ap()
        dbg["c0i"] = nc.dram_tensor("dbg_c0i", [128, D], F32,
                                    kind="ExternalOutput").ap()
        dbg["x0"] = nc.dram_tensor("dbg_x0", [4, 128, 4, 256], BF16,
                                   kind="ExternalOutput").ap()
        dbg["h0T1"] = nc.dram_tensor("dbg_h0T1", [128, KC, 128], BF16,
                                     kind="ExternalOutput").ap()
        dbg["c01"] = nc.dram_tensor("dbg_c01", [128, D], F32,
                                    kind="ExternalOutput").ap()
        dbg["h1T1"] = nc.dram_tensor("dbg_h1T1", [128, KC, 128], BF16,
                                     kind="ExternalOutput").ap()
        dbg["c11"] = nc.dram_tensor("dbg_c11", [128, D], F32,
                                    kind="ExternalOutput").ap()

    with tile.TileContext(nc) as tc:
        build_body(nc, tc, nsteps,
                   d_zT, d_zTs, d_gT, d_bT,
                   d_wih0, d_whh0, d_wih1_8, d_wih1_g, d_whh1, d_wlin,
                   d_b0, d_b1, d_blin, d_out, dbg)
    nc.compile()
    return nc


def build_body(nc, tc, nsteps, d_zT, d_zTs, d_gT, d_bT,
               d_wih0, d_whh0, d_wih1_8, d_wih1_g, d_whh1, d_wlin,
               d_b0, d_b1, d_blin, d_out, dbg):
    # ---- whole-life SBUF ------------------------------------------------
    life = tc.alloc_tile_pool(name="life", bufs=1)
    whh0 = life.tile([128, KC, H4], FP8, tag="whh0")
    whh1 = life.tile([128, KC, H4], FP8, tag="whh1")
    wih1_8 = life.tile([128, KC, 3 * D], FP8, tag="wih1_8")
    c0 = life.tile([128, D], F32, tag="c0")
    c1 = life.tile([128, D], F32, tag="c1")
    h0T = life.tile([128, KC, 128], BF16, tag="h0T")
    h0T8 = life.tile([128, KC, 128], FP8, tag="h0T8")
    x0sb = life.tile([128, NG, 4, GS], BF16, tag="x0sb")
    idbf = life.tile([128, 128], BF16, tag="idbf")

    h1Tp = tc.alloc_tile_pool(name="h1Tp", bufs=2)
    h1T8p = tc.alloc_tile_pool(name="h1T8p", bufs=2)
    h1T8_init = h1T8p.tile([128, KC, 128], FP8, tag="h1T8", name="h1T8_init")

    make_identity(nc, idbf)

    # ---- INIT phase: BN stats + zn^T + c0/c1 + x0_proj ------------------
    with tc.tile_pool(name="initp", bufs=8) as initp, \
         tc.tile_pool(name="wkp", bufs=3) as wkp, \
         tc.tile_pool(name="ismall", bufs=1) as ismall, \
         tc.tile_pool(name="ipsum", bufs=4, space="PSUM") as ipsum:

        # -- DMA issue order: stats inputs, then weights by first use.
        # Big weights go on the Sync queue; the wk stream and lower-priority
        # loads go on the GpSimd queue so neither blocks the other. --
        zs_all = ismall.tile([128, KC, BS], F32, tag="zs")
        zs_b = bass.AP(tensor=d_zTs.tensor, offset=d_zTs.offset,
                       ap=[[BS, 128], [128 * BS, KC], [1, BS]])
        nc.sync.dma_start(out=zs_all, in_=zs_b)
        gT = ismall.tile([128, KC], F32, tag="gT")
        nc.sync.dma_start(out=gT, in_=d_gT)
        bT = ismall.tile([128, KC], F32, tag="bT")
        nc.sync.dma_start(out=bT, in_=d_bT)
        zts = []
        for k in range(KC):
            zt = initp.tile([128, B], BF16, tag="zt", name=f"zt{k}")
            nc.sync.dma_start(out=zt, in_=d_zT[k * 128:(k + 1) * 128, :])
            zts.append(zt)
        for k in range(KC):
            nc.sync.dma_start(out=whh0[:, k, :],
                              in_=d_whh0[k * 128:(k + 1) * 128, :])
        b0rep = ismall.tile([128, 16, 256], BF16, tag="b0rep")
        b0_b = bass.AP(tensor=d_b0.tensor, offset=d_b0.offset,
                       ap=[[0, 128], [1, H4]])
        nc.sync.dma_start(out=b0rep, in_=b0_b)

        eps_t = ismall.tile([128, 1], F32, tag="eps")
        nc.vector.memset(eps_t, EPS)
        znf = ismall.tile([128, KC, 128], F32, tag="znf")
        idf32 = ismall.tile([128, 128], F32, tag="idf32")
        make_identity(nc, idf32)

        st = ismall.tile([128, KC, 2, 6], F32, tag="st")
        mv = ismall.tile([128, KC, 2], F32, tag="mv")
        sd8 = ismall.tile([128, KC], F32, tag="sd8")
        rs8 = ismall.tile([128, KC], F32, tag="rs8")
        sc8 = ismall.tile([128, KC], F32, tag="sc8")

        for k in range(KC):
            nc.vector.bn_stats(out=st[:, k, 0, :], in_=zts[k][:, 0:512])
            nc.vector.bn_stats(out=st[:, k, 1, :], in_=zts[k][:, 512:1024])
            nc.vector.bn_aggr(out=mv[:, k, :], in_=st[:, k])
        nc.scalar.activation(out=sd8, in_=mv[:, :, 1:2], func=AF.Sqrt,
                             bias=eps_t)
        nc.vector.reciprocal(out=rs8, in_=sd8)
        nc.vector.tensor_mul(sc8, gT, rs8)

        for k in range(KC):
            # zn^T chunk (fp32): (z - mean) * scale + beta
            nc.vector.tensor_scalar(
                out=znf[:, k, :], in0=zs_all[:, k, :],
                scalar1=mv[:, k, 0:1], scalar2=sc8[:, k:k + 1],
                op0=ALU.subtract, op1=ALU.mult)
            nc.vector.tensor_scalar_add(znf[:, k, :], znf[:, k, :],
                                        bT[:, k:k + 1])
            # bf16 copy for matmul lhsT (h0 initial state) + fp8 x32 copies
            # (fp8 must be produced from fp32 — bf16->fp8 converts are broken)
            nc.scalar.copy(out=h0T[:, k, :], in_=znf[:, k, :])
            nc.vector.tensor_scalar_mul(h0T8[:, k, :], znf[:, k, :], HS)
            nc.vector.tensor_scalar_mul(h1T8_init[:, k, :], znf[:, k, :], HS)

        if dbg:
            nc.sync.dma_start(out=dbg["znT"], in_=znf)

        # x0_proj = zn @ W_ih0^T + (b_ih0 + b_hh0), group-major bf16.
        # W_ih0 streamed once; all 4 group PSUMs live (8 banks).
        psg = [ipsum.tile([128, 4, GS], F32, tag="ips", name=f"ips{g}")
               for g in range(NG)]
        for k in range(KC):
            wk = wkp.tile([128, H4], BF16, tag="wi0")
            nc.sync.dma_start(out=wk, in_=d_wih0[k * 128:(k + 1) * 128, :])
            for g in range(NG):
                for q in range(4):
                    nc.tensor.matmul(
                        psg[g][:, q, :], h0T[:, k, :],
                        wk[:, q * D + g * GS:q * D + (g + 1) * GS],
                        start=(k == 0 and q in (0, 2)),
                        stop=(k == KC - 1),
                        skip_group_check=True)
        for g in range(NG):
            nc.vector.tensor_add(x0sb[:, g], psg[g], b0rep[:, 4 * g:4 * g + 4, :])
            if dbg:
                nc.sync.dma_start(out=dbg["x0"][g], in_=x0sb[:, g])

        # whh1/wih1_8 issued after the wk stream so x0_proj isn't delayed;
        # they are needed ~10us after the last wk chunk (step 0's layer 1)
        for k in range(KC):
            nc.sync.dma_start(out=whh1[:, k, :],
                              in_=d_whh1[k * 128:(k + 1) * 128, :])
        for k in range(KC):
            nc.sync.dma_start(out=wih1_8[:, k, :],
                              in_=d_wih1_8[k * 128:(k + 1) * 128, :])

        # c0 = c1 = zn in [b, d] layout via PE transpose of fp32 zn^T
        # (after x0proj so the transposes don't stall the PE early)
        for k in range(KC):
            pt = ipsum.tile([128, 4, GS], F32, tag="ips", name=f"tpz{k}")
            nc.tensor.transpose(pt[:, 0, 0:128], znf[:, k, :], idf32)
            nc.scalar.copy(out=c0[:, k * 128:(k + 1) * 128], in_=pt[:, 0, 0:128])
        nc.vector.tensor_copy(out=c1, in_=c0)
        if dbg:
            nc.sync.dma_start(out=dbg["c0i"], in_=c0)

    # ---- post-init loads (reuse init SBUF space) ------------------------
    post = tc.alloc_tile_pool(name="post", bufs=1)
    b1rep = post.tile([128, 16, 256], BF16, tag="b1rep")
    b1_b = bass.AP(tensor=d_b1.tensor, offset=d_b1.offset,
                   ap=[[0, 128], [1, H4]])
    nc.sync.dma_start(out=b1rep, in_=b1_b)
    blin = post.tile([128, D], F32, tag="blin")
    blin_b = bass.AP(tensor=d_blin.tensor, offset=d_blin.offset,
                     ap=[[0, 128], [1, D]])
    nc.sync.dma_start(out=blin, in_=blin_b)
    wih1_g = post.tile([128, KC, D], BF16, tag="wih1_g")
    for k in range(KC):
        nc.sync.dma_start(out=wih1_g[:, k, :],
                          in_=d_wih1_g[k * 128:(k + 1) * 128, :])
    wlin = post.tile([128, KC, D], BF16, tag="wlin")
    for k in range(KC):
        nc.sync.dma_start(out=wlin[:, k, :],
                          in_=d_wlin[k * 128:(k + 1) * 128, :])

    # ---- recurrent loop --------------------------------------------------
    with tc.tile_pool(name="gates", bufs=3, space="PSUM") as gpool, \
         tc.tile_pool(name="trp", bufs=1, space="PSUM") as trpool, \
         tc.tile_pool(name="yps", bufs=1, space="PSUM") as ypool, \
         tc.tile_pool(name="tmp", bufs=3) as tmp, \
         tc.tile_pool(name="hfp", bufs=2) as hfp, \
         tc.tile_pool(name="hst", bufs=4) as hst, \
         tc.tile_pool(name="ysp", bufs=2) as ysp:

        h1T8 = h1T8_init

        def cell_math(g, ps, c, x0_slice, hs):
            # gates: ps[:, 0..3, :] = i, f, o, g (pre-activation x GSC, fp32)
            # psum + sbuf -> sbuf: releases the PSUM tile after one DVE op
            src = tmp.tile([128, 4, GS], F32, tag="sum")
            nc.vector.tensor_add(src, ps, x0_slice)
            sg = tmp.tile([128, 3, GS], BF16, tag="sg")
            nc.scalar.activation(out=sg, in_=src[:, 0:3, :], func=AF.Sigmoid,
                                 scale=INV_GSC)
            tg = tmp.tile([128, GS], BF16, tag="tg")
            nc.scalar.activation(out=tg, in_=src[:, 3, :], func=AF.Tanh,
                                 scale=INV_GSC)
            csl = c[:, g * GS:(g + 1) * GS]
            nc.vector.tensor_mul(csl, csl, sg[:, 1, :])          # c *= sig(f)
            tp = tmp.tile([128, GS], F32, tag="tp")
            nc.vector.tensor_mul(tp, sg[:, 0, :], tg)            # sig(i)*tanh(g)
            nc.vector.tensor_add(csl, csl, tp)
            tc2 = tmp.tile([128, GS], BF16, tag="tc2")
            nc.scalar.activation(out=tc2, in_=csl, func=AF.Tanh)
            nc.vector.tensor_mul(hs, sg[:, 2, :], tc2)           # h = sig(o)*tanh(c)

        def hh_matmuls(ps, hT8, w, g):
            for kp in range(KC // 2):
                for q in range(4):
                    nc.tensor.matmul(
                        ps[:, q, :], hT8[:, 2 * kp:2 * kp + 2, :],
                        w[:, 2 * kp:2 * kp + 2,
                          q * D + g * GS:q * D + (g + 1) * GS],
                        start=(kp == 0 and q in (0, 2)),
                        stop=False,
                        perf_mode=PM.DoubleRow,
                        skip_group_check=True)

        def transpose_quad(hs_pair, hT, half):
            # hs_pair: two [128, 256] h slices -> hT[:, 4*half:4*half+4, :]
            trq = trpool.tile([128, 512], BF16, tag="tr")
            for j in range(2):
                nc.tensor.transpose(trq[:, 128 * j:128 * (j + 1)],
                                    hs_pair[0][:, 128 * j:128 * (j + 1)], idbf)
                nc.tensor.transpose(trq[:, 256 + 128 * j:256 + 128 * (j + 1)],
                                    hs_pair[1][:, 128 * j:128 * (j + 1)], idbf)
            nc.scalar.copy(out=hT[:, 4 * half:4 * half + 4, :], in_=trq)

        ps0_g0 = None
        for t in range(nsteps):
            # --- layer 0: gates0 = x0_proj + h0 @ W_hh0^T (fp8 DoubleRow) ---
            # (group 0's matmuls were already emitted at the end of step t-1
            # to fill the PE while cell1-g3's Act/DVE chain completes)
            hs0 = []
            for g in range(NG):
                if g == 0 and ps0_g0 is not None:
                    ps = ps0_g0
                else:
                    ps = gpool.tile([128, 4, GS], F32, tag="g")
                    hh_matmuls(ps, h0T8, whh0, g)
                # close the accumulation for the sim on the last writes
                hs = hst.tile([128, GS], BF16, tag="h")
                hs0.append(hs)
                cell_math(g, ps, c0, x0sb[:, g], hs)

            # --- layer 1: hh first (uses h1T8 from t-1), h0 transposes
            # pipelined in halves across the hh window, then ih ---
            hf = hfp.tile([128, KC, 128], F32, tag="hf")
            ps1 = []
            for g in range(2):
                ps = gpool.tile([128, 4, GS], F32, tag="g")
                ps1.append(ps)
                hh_matmuls(ps, h1T8, whh1, g)
            # first half of h0: transposes, upcast, fp8 convert (pipelined
            # so the h0T8 chain isn't serialized behind g3's cell math)
            transpose_quad(hs0[0:2], h0T, 0)
            nc.scalar.copy(out=hf[:, 0:4, :], in_=h0T[:, 0:4, :])
            nc.vector.tensor_scalar_mul(h0T8[:, 0:4, :], hf[:, 0:4, :], HS)
            for g in range(2, 4):
                ps = gpool.tile([128, 4, GS], F32, tag="g")
                ps1.append(ps)
                hh_matmuls(ps, h1T8, whh1, g)
            # second half (L0 g3 cell math has finished by now)
            transpose_quad(hs0[2:4], h0T, 1)
            nc.scalar.copy(out=hf[:, 4:8, :], in_=h0T[:, 4:8, :])
            nc.vector.tensor_scalar_mul(h0T8[:, 4:8, :], hf[:, 4:8, :], HS)

            h1T_new = h1Tp.tile([128, KC, 128], BF16, tag="h1T")
            h1T8_new = h1T8p.tile([128, KC, 128], FP8, tag="h1T8")
            hs1 = []
            for g in range(NG):
                ps = ps1[g]
                # i,f,o gate columns: fp8 DoubleRow against h0T8
                for kp in range(KC // 2):
                    for q in range(3):
                        nc.tensor.matmul(
                            ps[:, q, :], h0T8[:, 2 * kp:2 * kp + 2, :],
                            wih1_8[:, 2 * kp:2 * kp + 2,
                                   q * D + g * GS:q * D + (g + 1) * GS],
                            start=False,
                            stop=(kp == KC // 2 - 1),
                            perf_mode=PM.DoubleRow,
                            skip_group_check=True)
                # g gate columns: bf16 against h0T
                for k in range(KC):
                    nc.tensor.matmul(
                        ps[:, 3, :], h0T[:, k, :],
                        wih1_g[:, k, g * GS:(g + 1) * GS],
                        start=False,
                        stop=(k == KC - 1),
                        skip_group_check=True)
                hs = hst.tile([128, GS], BF16, tag="h")
                hs1.append(hs)
                cell_math(g, ps, c1, b1rep[:, 4 * g:4 * g + 4, :], hs)
                if g == 1:
                    transpose_quad(hs1[0:2], h1T_new, 0)
            # emit next step's L0 g0 matmuls now: h0T8 already holds h0(t),
            # and this fills the PE while cell1-g3's chain produces hs1[3]
            if t + 1 < nsteps:
                ps0_g0 = gpool.tile([128, 4, GS], F32, tag="g")
                hh_matmuls(ps0_g0, h0T8, whh0, 0)
            else:
                ps0_g0 = None
            transpose_quad(hs1[2:4], h1T_new, 1)

            # --- inline final linear: y_t = h1_t @ W_lin^T + b_lin ---
            ys = ysp.tile([128, D], F32, tag="ys")
            for cch in range(4):
                yp = ypool.tile([128, 256], F32, tag="y")
                for k in range(KC):
                    nc.tensor.matmul(
                        yp, h1T_new[:, k, :],
                        wlin[:, k, cch * 256:(cch + 1) * 256],
                        start=(k == 0), stop=(k == KC - 1),
                        skip_group_check=True)
                nc.vector.tensor_add(ys[:, cch * 256:(cch + 1) * 256], yp,
                                     blin[:, cch * 256:(cch + 1) * 256])
            nc.sync.dma_start(out=d_out[:, t, :], in_=ys)

            # h1 fp8 copy for next step's L1 hh (not latency-critical: runs
            # on Act/DVE while the PE does y and the next step's L0)
            hf1 = hfp.tile([128, KC, 128], F32, tag="hf")
            nc.scalar.copy(out=hf1, in_=h1T_new)
            nc.vector.tensor_scalar_mul(h1T8_new, hf1, HS)

            if dbg and t == 0:
                nc.sync.dma_start(out=dbg["h0T1"], in_=h0T)
                nc.sync.dma_start(out=dbg["c01"], in_=c0)
                nc.sync.dma_start(out=dbg["h1T1"], in_=h1T_new)
                nc.sync.dma_start(out=dbg["c11"], in_=c1)
            h1T8 = h1T8_new

    post.release()
    h1T8p.release()
    h1Tp.release()
    life.release()


_CACHE = {}


def _get_module(nsteps=BAR):
    if nsteps not in _CACHE:
        _CACHE[nsteps] = build_module(nsteps)
    return _CACHE[nsteps]


def prep_inputs(z, bn_gamma, bn_beta, W_ih0, W_hh0, b_ih0, b_hh0,
                W_ih1, W_hh1, b_ih1, b_hh1, W_lin, b_lin):
    z = np.asarray(z, np.float32)
    zT = np.ascontiguousarray(z.T)
    wih1p = _gate_perm(np.asarray(W_ih1, np.float32).T)  # [D, 4, D] i,f,o,g
    common = {
        "zT": _np_bf16(zT),
        "gammaT": np.ascontiguousarray(
            np.asarray(bn_gamma, np.float32).reshape(KC, 128).T),
        "betaT": np.ascontiguousarray(
            np.asarray(bn_beta, np.float32).reshape(KC, 128).T),
        # x2048 prescale on bf16 so gate contributions match the fp8 (x32*x64)
        # PSUM scale; activations divide it back out.
        "wt_ih0": _np_bf16(_gate_perm(np.asarray(W_ih0, np.float32).T)
                           .reshape(D, H4) * GSC),
        "wt_hh0": _np_fp8(_gate_perm(np.asarray(W_hh0, np.float32).T)
                          .reshape(D, H4) * WS),
        "wt_ih1_8": _np_fp8(wih1p[:, 0:3, :].reshape(D, 3 * D) * WS),
        "wt_ih1_g": _np_bf16(wih1p[:, 3, :] * GSC),
        "wt_hh1": _np_fp8(_gate_perm(np.asarray(W_hh1, np.float32).T)
                          .reshape(D, H4) * WS),
        "wt_lin": _np_bf16(np.asarray(W_lin, np.float32).T),
        "b0r": _np_bf16(GSC * _gate_bias(np.asarray(b_ih0, np.float32)
                                         + np.asarray(b_hh0, np.float32))
                        .reshape(H4)),
        "b1r": _np_bf16(GSC * _gate_bias(np.asarray(b_ih1, np.float32)
                                         + np.asarray(b_hh1, np.float32))
                        .reshape(H4)),
        "b_lin": np.asarray(b_lin, np.float32),
    }
    in_maps = []
    for c in range(NCORES):
        m = dict(common)
        m["zTs"] = np.ascontiguousarray(zT[:, c * BS:(c + 1) * BS])
        in_maps.append(m)
    return in_maps


def kernel(**inputs):
    nc = _get_module()
    in_maps = prep_inputs(**inputs)
    res = bass_utils.run_bass_kernel_spmd(nc, in_maps, core_ids=list(range(NCORES)))
    out = np.concatenate([res.results[c]["out"] for c in range(NCORES)], axis=0)
    return out.astype(np.float32)
